# revision 15
# baseline (speedup 1.0000x reference)
"""Trainium2 Bass kernel for chunked self-attention feature map.

Reference computation (per 32x32 spatial chunk, 512 chunks total):
  q = Wq@xc + bq;  k = maxpool2(Wk@xc + bk);  v = maxpool2(Wv@xc + bv)
  attn = softmax(q^T k over k-axis);  ao = v @ attn^T
  out = gamma*(Wo@ao + bo) + xc

Sharding: data-parallel over N (batch) -- core i processes x[i] (64 chunks),
no cross-device communication.

Algebraic folds (host-side):
  - bk is softmax-invariant (shifts all k-logits by a per-q constant) -> dropped.
  - bq folded into q via DVE tensor_scalar_add on PSUM evacuation.
  - bv contributes Wo@bv per output channel (sum_k attn = 1) -> folded, with
    gamma and bo, into a per-channel bias applied by the ACT epilogue.
  - softmax denominator from a ones-column in the v^T matmul operand;
    division delayed to the 32-partition `ao` evacuation.
  - softmax max-subtraction skipped: logits are ~N(0, 3.5), |logit| < 30 << 88.
  - residual +xc via identity matmul accumulated into the output PSUM.

Precision strategy (fp32 matmul is 4 cyc/col on TRN2; f32r/bf16 are 1):
  - conv / energy / output matmuls in float32r (~13-bit mantissa, rel 2.4e-4).
    f32r operands MUST be produced by SWDGE dma casts or DVE writes; f32r
    weights (lhsT) read as zeros unless DMA-produced, and ACT cannot write
    f32r at all (device-fatal) -- both hit on hardware.
  - exp/ao matmul in bf16; the softmax denominator is computed from the same
    bf16 exps, so quantization largely cancels in attn = exp/denom.
  - PSUM accumulation and the reciprocal path stay fp32.
  - 1/denom = exp(-ln(denom)) on ACT: DVE exact reciprocal is ~8 cyc/elem and
    reciprocal_approx_fast mis-computes on this hardware.
"""

import sys

if '/opt/trn_rl_repo' not in sys.path:
    sys.path.insert(0, '/opt/trn_rl_repo')

from contextlib import ExitStack

import numpy as np

import concourse.bass as bass
import concourse.mybir as mybir
import concourse.tile as tile
from concourse import bacc
from concourse.bass import ts
from concourse.bass_utils import run_bass_kernel_spmd
from concourse.masks import make_identity

F32 = mybir.dt.float32
F32R = mybir.dt.float32r
BF16 = mybir.dt.bfloat16
AF = mybir.ActivationFunctionType

N_CORES = 8
C = 256          # input/output channels
CH = 32          # inner attention channels
CHH = CHW = 32   # chunk spatial size
NQ = CHH * CHW   # 1024 query positions per chunk
NKV = NQ // 4    # 256 kv positions per chunk


def build_nc(ngy=8, nhx=2):
    """Build the per-core Bass program.

    Per core: x shard [256, 32*ngy, 128*nhx]; ngy*nhx slabs of 4 chunks.
    """
    H = CHH * ngy
    W = 128 * nhx
    nc = bacc.Bacc("TRN2", target_bir_lowering=False, debug=False)
    xd = nc.dram_tensor("x", [C, H, W], F32, kind="ExternalInput")
    w1d = nc.dram_tensor("w1", [2, 128, 96], F32, kind="ExternalInput")
    lod = nc.dram_tensor("lo", [CH, C], F32, kind="ExternalInput")
    bqd = nc.dram_tensor("bq", [CH, 1], F32, kind="ExternalInput")
    beffd = nc.dram_tensor("beff", [2, 128, 1], F32, kind="ExternalInput")
    identd = nc.dram_tensor("ident", [128, 128], F32, kind="ExternalInput")
    yd = nc.dram_tensor("y", [C, H, W], F32, kind="ExternalOutput")

    with tile.TileContext(nc) as tc, ExitStack() as ctx:
        const = ctx.enter_context(tc.tile_pool(name="const", bufs=1))
        xin_p = ctx.enter_context(tc.tile_pool(name="xin", bufs=2))
        yout_p = ctx.enter_context(tc.tile_pool(name="yout", bufs=2))
        work = ctx.enter_context(tc.tile_pool(name="work", bufs=2))
        exps = ctx.enter_context(tc.tile_pool(name="exps", bufs=2))
        ps = ctx.enter_context(tc.tile_pool(name="ps", bufs=4, space="PSUM"))

        # constants / weights (f32r matmul operands come from SWDGE dma casts)
        w1sb = const.tile([128, 2, 96], F32R)      # packed [WqT|WkT|WvT], C split
        nc.gpsimd.dma_start(w1sb[:], w1d.rearrange("k p m -> p k m"))
        losb = const.tile([CH, C], F32R)           # (gamma*Wo)^T
        nc.gpsimd.dma_start(losb[:], lod[:])
        bqsb = const.tile([CH, 1], F32)
        nc.sync.dma_start(bqsb[:], bqd[:])
        beffsb = const.tile([128, 2, 1], F32)      # gamma*(bo + Wo@bv), C split
        nc.sync.dma_start(beffsb[:], beffd.rearrange("k p o -> p k o"))
        i128 = const.tile([128, 128], F32R)
        nc.gpsimd.dma_start(i128[:], identd[:])
        i32 = const.tile([32, 32], F32)
        make_identity(nc, i32[:])

        def maxpool(src_ap, tag, out_dt):
            # src_ap: [32, 1024] psum with layout h*32+w -> [32, 256] (i*16+j)
            # stage 1 is a reduce (single input): one DVE PSUM read port.
            tmp = work.tile([CH, 32, 16], F32, tag="pool_tmp")
            s = src_ap.rearrange("p (hw2 t) -> p hw2 t", t=2)
            nc.vector.tensor_reduce(
                tmp.rearrange("p h w2 -> p (h w2)"), s,
                axis=mybir.AxisListType.X, op=mybir.AluOpType.max)
            out = work.tile([CH, NKV], out_dt, tag=tag)
            t2 = tmp.rearrange("p (i t) w2 -> p i t w2", t=2)
            ov = out.rearrange("p (i w2) -> p i w2", i=16)
            nc.vector.tensor_max(ov[:], t2[:, :, 0, :], t2[:, :, 1, :])
            return out

        def chunk(xins, youts, j):
            def xc_ap(kc, nh):
                return xins[kc][:, 16 * nh:16 * (nh + 1), 32 * j:32 * (j + 1)]

            # fused q/k/v 1x1 conv: [96, 1024] psum
            pqkv = ps.tile([96, NQ], F32, tag="ps")
            for nh in range(2):
                for kc in range(2):
                    nc.tensor.matmul(
                        pqkv[:, ts(nh, 512)], w1sb[:, kc, :], xc_ap(kc, nh),
                        start=(kc == 0), stop=(kc == 1))

            # evacuate q (+bq) to sbuf as f32r (energy rhs)
            qf = work.tile([CH, NQ], F32R, tag="qf")
            nc.vector.tensor_scalar_add(qf[:], pqkv[0:CH, :], bqsb[:])

            # 2x2 maxpool k and v
            kpf = maxpool(pqkv[32:64, :], "kpf", F32)
            vp = maxpool(pqkv[64:96, :], "vp", F32)
            # energy lhsT must be a DMA-produced f32r tile (DVE-written f32r
            # weights load as zeros on this hardware)
            kp = work.tile([CH, NKV], F32R, tag="kp")
            nc.gpsimd.dma_start(kp[:], kpf[:])

            # v^T via PE transpose (fp32, exact); lhsT for bf16 ao matmul,
            # col 32 = ones (denominator accumulator)
            vpt = ps.tile([128, 2, 32], F32, tag="ps")
            for b in range(2):
                nc.tensor.transpose(vpt[:, b, :], vp[:, ts(b, 128)], i32[:])
            lao = work.tile([128, 2, 33], BF16, tag="lao")
            nc.vector.tensor_copy(lao[:, :, 0:32], vpt[:])
            nc.gpsimd.memset(lao[:, :, 32:33], 1.0)

            # energy^T = kp^T @ qf : [256, 1024] as two [128, 1024] psum tiles
            ets = [ps.tile([128, NQ], F32, tag="ps", name=f"et{m}")
                   for m in range(2)]
            for m in range(2):
                for nh in range(2):
                    nc.tensor.matmul(ets[m][:, ts(nh, 512)], kp[:, ts(m, 128)],
                                     qf[:, ts(nh, 512)], start=True, stop=True)

            # exp -> bf16 (no max subtraction needed; logits are small)
            xts = []
            for m in range(2):
                xt = exps.tile([128, NQ], BF16, tag=f"exp{m}", name=f"expt{m}")
                nc.scalar.activation(xt[:], ets[m][:], AF.Exp)
                xts.append(xt)

            # ao_unnorm (rows 0-31) + denominator (row 32), bf16 matmul
            ao = ps.tile([33, NQ], F32, tag="ps")
            for nh in range(2):
                for kc in range(2):
                    nc.tensor.matmul(ao[:, ts(nh, 512)], lao[:, kc, :],
                                     xts[kc][:, ts(nh, 512)],
                                     start=(kc == 0), stop=(kc == 1))

            # 1/denom via exp(-ln(denom)) on ACT; broadcast to 32 partitions
            lnd = work.tile([1, NQ], F32, tag="lnd")
            nc.scalar.activation(lnd[:], ao[32:33, :], AF.Ln)
            recip = work.tile([1, NQ], F32, tag="recip")
            nc.scalar.activation(recip[:], lnd[:], AF.Exp, scale=-1.0)
            r32 = work.tile([32, NQ], F32, tag="r32")
            nc.gpsimd.partition_broadcast(r32[:], recip[:])
            aon = work.tile([CH, NQ], F32R, tag="aon")
            nc.vector.tensor_mul(aon[:], ao[0:32, :], r32[:])

            # out = (g*Wo)@aon + xc; +g*(bo+Wo@bv) via epilogue ACT bias
            for m in range(2):
                ot = ps.tile([128, NQ], F32, tag="ps", name=f"ot{m}")
                for nh in range(2):
                    nc.tensor.matmul(ot[:, ts(nh, 512)], losb[:, ts(m, 128)],
                                     aon[:, ts(nh, 512)], start=True, stop=False)
                    nc.tensor.matmul(ot[:, ts(nh, 512)], i128[:], xc_ap(m, nh),
                                     start=False, stop=True)
                ov = ot.rearrange("p (h w) -> p h w", w=32)
                nc.scalar.activation(youts[m][:, :, 32 * j:32 * (j + 1)], ov[:],
                                     AF.Identity, bias=beffsb[:, m, :])

        for gy in range(ngy):
            for hx in range(nhx):
                xins, youts = [], []
                for cb in range(2):
                    xt = xin_p.tile([128, CHH, 128], F32R,
                                    tag=f"xin{cb}", name=f"xin{cb}")
                    nc.gpsimd.dma_start(
                        xt[:], xd[128 * cb:128 * (cb + 1),
                                  CHH * gy:CHH * (gy + 1),
                                  128 * hx:128 * (hx + 1)])
                    xins.append(xt)
                    youts.append(yout_p.tile([128, CHH, 128], F32,
                                             tag=f"yout{cb}", name=f"yout{cb}"))
                for j in range(4):
                    chunk(xins, youts, j)
                for cb in range(2):
                    nc.sync.dma_start(
                        yd[128 * cb:128 * (cb + 1),
                           CHH * gy:CHH * (gy + 1),
                           128 * hx:128 * (hx + 1)], youts[cb][:])
    nc.compile()
    return nc


def pack_weights(Wq, bq, Wk, bk, Wv, bv, Wo, bo, gamma):
    g = np.float32(gamma[0])
    w1 = np.concatenate([Wq.T, Wk.T, Wv.T], axis=1).astype(np.float32)
    w1 = np.ascontiguousarray(w1.reshape(2, 128, 96))
    lo = np.ascontiguousarray((g * Wo).T.astype(np.float32))          # [32, 256]
    beff = (g * (bo + Wo @ bv)).astype(np.float32).reshape(2, 128, 1)
    bq2 = np.ascontiguousarray(bq.reshape(CH, 1)).astype(np.float32)
    ident = np.eye(128, dtype=np.float32)
    return w1, lo, bq2, np.ascontiguousarray(beff), ident


_NC_CACHE = {}


def _get_nc(key=(8, 2)):
    if key not in _NC_CACHE:
        _NC_CACHE[key] = build_nc(*key)
    return _NC_CACHE[key]


def kernel(x, Wq, bq, Wk, bk, Wv, bv, Wo, bo, gamma):
    x = np.asarray(x, dtype=np.float32)
    w1, lo, bq2, beff, ident = pack_weights(
        np.asarray(Wq), np.asarray(bq), np.asarray(Wk), np.asarray(bk),
        np.asarray(Wv), np.asarray(bv), np.asarray(Wo), np.asarray(bo),
        np.asarray(gamma))
    nc = _get_nc()
    in_maps = [
        {"x": np.ascontiguousarray(x[i]), "w1": w1, "lo": lo, "bq": bq2,
         "beff": beff, "ident": ident}
        for i in range(N_CORES)
    ]
    res = run_bass_kernel_spmd(nc, in_maps, list(range(N_CORES)))
    return np.stack([res.results[i]["y"] for i in range(N_CORES)], axis=0)
